# revision 1
# baseline (speedup 1.0000x reference)
"""MoE expert-group kernel for 8 Trainium2 NeuronCores.

Strategy (expert-parallel, per the sharding hint):
  - Host computes the (tiny) router: logits = x @ Wg.T, top-2, softmax.
  - Tokens are gathered per expert on host ("dispatch"); core c owns
    experts (2c, 2c+1) and receives its two experts' tokens (transposed,
    zero-padded to a uniform capacity) plus its two experts' weights.
  - Each core runs a dense 2-layer MLP (relu(x@W1+b1)@W2+b2) over its
    gathered tokens in transposed layout: weights are the stationary
    matmul operand in their natural [in, out] layout, activations stream
    as the moving operand, biases become per-partition activation biases.
  - Host applies the per-(token, expert) softmax weight and scatter-adds
    ("combine") back to the full [8192, 1024] output, in the same expert
    order as the reference loop.

Only the dense MLP FLOPs (the compute-bound part, 1/8 of the dense-all-
experts reference) run on device; routing/gather/combine are O(N*E) or
O(N*D) host work.
"""

import os
import sys
import time

import numpy as np

sys.path.insert(0, "/opt/trn_rl_repo")

N_TOKENS = 8192
D_MODEL = 1024
D_HIDDEN = 2048
N_EXPERTS = 16
TOP_K = 2
N_CORES = 8
EPC = N_EXPERTS // N_CORES  # experts per core
KC1 = D_MODEL // 128   # k-chunks layer 1
MC1 = D_HIDDEN // 128  # m-chunks layer 1
KC2 = D_HIDDEN // 128  # k-chunks layer 2
MC2 = D_MODEL // 128   # m-chunks layer 2

# matmul dtype mode: "fp32" (bit-exact-ish, 1/4 rate), "fp32r" (full rate,
# reduced-precision multiplies), "bf16" (full rate, bf16 operands)
MM_MODE = os.environ.get("KERNEL_MM_MODE", "fp32r")


def _split_tiles(cap):
    """Split cap (multiple of 128) into moving-dim tiles <=512, all >=256
    when possible (fp32r runs full-rate only for moving dim >= 256)."""
    if cap % 384 == 0:
        return [384] * (cap // 384)
    tiles = []
    r = cap
    while r > 512:
        take = 512 if (r - 512 == 0 or r - 512 >= 256) else 384
        tiles.append(take)
        r -= take
    tiles.append(r)
    # ascending: the last (largest) tile maximizes the compute window that
    # hides the next expert's W2-half prefetch
    return sorted(tiles)


def build_program(cap, mode=MM_MODE, loop_reps=1, pipe=None, fine_chunks=None):
    """Build the per-core program. loop_reps>1 wraps the body in a hardware
    For_i loop (identical work each iteration) for wall-clock timing."""
    import contextlib

    import concourse.mybir as mybir
    import concourse.tile as tile
    from concourse import bacc

    f32 = mybir.dt.float32
    # Matmul-operand dtypes are carried end-to-end (DRAM and SBUF): the BIR
    # verifier requires fp32r matmul inputs to be *produced* as fp32r, so
    # the whole operand chain carries the dtype. act_dt covers the moving
    # operands (x, h), w_dt the stationary weights.
    if mode == "fp32":
        act_dt = w_dt = f32
    elif mode == "fp32r":
        act_dt = w_dt = mybir.dt.float32r
    elif mode == "bf16":
        act_dt = w_dt = mybir.dt.bfloat16
    elif mode == "wbf16":
        act_dt = mybir.dt.float32r
        w_dt = mybir.dt.bfloat16
    else:
        raise ValueError(mode)

    if pipe is None:
        pipe = os.environ.get("KERNEL_PIPE", "1") == "1"
    if fine_chunks is None:
        fine_chunks = os.environ.get("KERNEL_FINE", "1") == "1"

    tiles = _split_tiles(cap)

    nc = bacc.Bacc("TRN2", target_bir_lowering=False, debug=False)
    xt = nc.dram_tensor("xt", [EPC, D_MODEL, cap], act_dt, kind="ExternalInput").ap()
    w1 = nc.dram_tensor("w1", [EPC, D_MODEL, D_HIDDEN], w_dt, kind="ExternalInput").ap()
    b1 = nc.dram_tensor("b1", [EPC, D_HIDDEN], f32, kind="ExternalInput").ap()
    w2 = nc.dram_tensor("w2", [EPC, D_HIDDEN, D_MODEL], w_dt, kind="ExternalInput").ap()
    b2 = nc.dram_tensor("b2", [EPC, D_MODEL], f32, kind="ExternalInput").ap()
    yt = nc.dram_tensor("yt", [EPC, D_MODEL, cap], f32, kind="ExternalOutput").ap()

    Relu = mybir.ActivationFunctionType.Relu
    Ident = mybir.ActivationFunctionType.Identity

    HALF = MC2 // 2  # m2 chunks per W2 half-pool
    with tile.TileContext(nc) as tc:
        with (
            tc.tile_pool(name="w1pa", bufs=1) as w1pa,
            tc.tile_pool(name="w1pb", bufs=1) as w1pb,
            tc.tile_pool(name="w2pa", bufs=1) as w2pa,
            tc.tile_pool(name="w2pb", bufs=1) as w2pb,
            tc.tile_pool(name="bp", bufs=2) as bp,
            tc.tile_pool(name="xp", bufs=2) as xp,
            tc.tile_pool(name="hp", bufs=2 if pipe else 1) as hp,
            tc.tile_pool(name="yp", bufs=4) as yp,
            tc.tile_pool(name="ps1", bufs=2, space="PSUM") as ps1,
            tc.tile_pool(name="ps2", bufs=2, space="PSUM") as ps2,
        ):
            loop_cm = (
                tc.For_i(0, loop_reps, 1)
                if loop_reps > 1
                else contextlib.nullcontext()
            )
            with loop_cm:
                for e in range(EPC):
                    xt_src = xt[e].rearrange("(c p) n -> p c n", p=128)
                    yt_dst = yt[e].rearrange("(c p) n -> p c n", p=128)

                    # The DMA fabric drains transfers roughly in enqueue
                    # order, so emissions follow need order. W1: two
                    # half-pools (m 0..7 / 8..15) x two quarter-DMAs each on
                    # sync; quarters keep startup short, half-pools release
                    # early so the next expert's W1 hides under compute.
                    # Biases slot between quarters (tiny, needed early).
                    w1_src = w1[e].rearrange("(c p) m -> p c m", p=128)
                    HW1 = D_HIDDEN // 2
                    QW = D_HIDDEN // 4
                    w1ta = w1pa.tile([128, KC1, HW1], w_dt, tag="w1ta")
                    w1tb = w1pb.tile([128, KC1, HW1], w_dt, tag="w1tb")
                    NQ1 = 4 if fine_chunks else 2  # DMAs per W1 half-pool
                    EW = HW1 // NQ1
                    nc.sync.dma_start(w1ta[:, :, :EW], w1_src[:, :, :EW])
                    b1t = bp.tile([128, MC1], f32, tag="b1t")
                    nc.sync.dma_start(b1t[:], b1[e].rearrange("(m p) -> p m", p=128))
                    b2t = bp.tile([128, MC2], f32, tag="b2t")
                    nc.sync.dma_start(b2t[:], b2[e].rearrange("(m p) -> p m", p=128))
                    for q in range(1, NQ1):
                        nc.sync.dma_start(
                            w1ta[:, :, q * EW : (q + 1) * EW],
                            w1_src[:, :, q * EW : (q + 1) * EW],
                        )
                    for q in range(NQ1):
                        nc.sync.dma_start(
                            w1tb[:, :, q * EW : (q + 1) * EW],
                            w1_src[:, :, HW1 + q * EW : HW1 + (q + 1) * EW],
                        )

                    # gpsimd queue in need order: xt[0], W2 quarters (into
                    # two half-pools, so the next expert's halves load under
                    # this expert's L2), then xt[1..].
                    w2_src = w2[e].rearrange("(c p) m -> p c m", p=128)
                    off = [sum(tiles[:j]) for j in range(len(tiles))]
                    xtiles = []
                    for j, nt in enumerate(tiles):
                        xtile_j = xp.tile([128, KC1, nt], act_dt, tag="xtile", name=f"xtile_{e}_{j}")
                        xtiles.append(xtile_j)
                    # xt[0] split by k-chunk pairs: the first L1 matmul only
                    # needs k-chunk 0, so it starts after ~0.5MB, not 1.5MB.
                    for cc in range(0, KC1, 2):
                        nc.gpsimd.dma_start(
                            xtiles[0][:, cc : cc + 2, :],
                            xt_src[:, cc : cc + 2, off[0] : off[0] + tiles[0]],
                        )
                    if len(tiles) > 1:
                        nc.gpsimd.dma_start(
                            xtiles[1][:], xt_src[:, :, off[1] : off[1] + tiles[1]]
                        )
                    HW2 = HALF * 128
                    w2ta = w2pa.tile([128, KC2, HW2], w_dt, tag="w2ta")
                    w2tb = w2pb.tile([128, KC2, HW2], w_dt, tag="w2tb")
                    NQ2 = 4 if fine_chunks else 2  # DMAs per W2 half-pool
                    QW2 = HW2 // NQ2
                    for q in range(NQ2):
                        nc.gpsimd.dma_start(
                            w2ta[:, :, q * QW2 : (q + 1) * QW2],
                            w2_src[:, :, q * QW2 : (q + 1) * QW2],
                        )
                    for q in range(NQ2):
                        nc.gpsimd.dma_start(
                            w2tb[:, :, q * QW2 : (q + 1) * QW2],
                            w2_src[:, :, HW2 + q * QW2 : HW2 + (q + 1) * QW2],
                        )
                    for j in range(2, len(tiles)):
                        nc.gpsimd.dma_start(
                            xtiles[j][:], xt_src[:, :, off[j] : off[j] + tiles[j]]
                        )

                    # j-level software pipeline: L1(0), L1(1), L2(0),
                    # L1(2), L2(1), ... — PE is in-order, so emitting the
                    # next tile's L1 before this tile's L2 lets layer-1 run
                    # while W2 is still streaming in (ht is double-buffered).
                    hts = [None] * len(tiles)

                    def layer1(j):
                        nt = tiles[j]
                        ht = hp.tile([128, KC2, nt], act_dt, tag="ht",
                                     name=f"ht_{e}_{j}")
                        hts[j] = ht
                        for m in range(MC1):
                            w1h = w1ta if m < MC1 // 2 else w1tb
                            mh = m % (MC1 // 2)
                            hps = ps1.tile([128, nt], f32, tag="hps")
                            for c in range(KC1):
                                nc.tensor.matmul(
                                    hps[:],
                                    lhsT=w1h[:, c, mh * 128 : (mh + 1) * 128],
                                    rhs=xtiles[j][:, c, :],
                                    start=(c == 0),
                                    stop=(c == KC1 - 1),
                                )
                            nc.scalar.activation(
                                ht[:, m, :], hps[:], Relu, bias=b1t[:, m : m + 1]
                            )

                    def layer2(j):
                        nt = tiles[j]
                        ht = hts[j]
                        for m in range(MC2):
                            w2h = w2ta if m < HALF else w2tb
                            mh = m % HALF
                            yps = ps2.tile([128, nt], f32, tag="yps")
                            for c in range(KC2):
                                nc.tensor.matmul(
                                    yps[:],
                                    lhsT=w2h[:, c, mh * 128 : (mh + 1) * 128],
                                    rhs=ht[:, c, :],
                                    start=(c == 0),
                                    stop=(c == KC2 - 1),
                                )
                            ysb = yp.tile([128, nt], f32, tag="ysb")
                            nc.scalar.activation(
                                ysb[:], yps[:], Ident, bias=b2t[:, m : m + 1]
                            )
                            nc.scalar.dma_start(
                                yt_dst[:, m, off[j] : off[j] + nt], ysb[:]
                            )

                    T = len(tiles)
                    if pipe:
                        for k in range(T + 1):
                            if k < T:
                                layer1(k)
                            if k >= 1:
                                layer2(k - 1)
                    else:
                        for k in range(T):
                            layer1(k)
                            layer2(k)
    nc.compile()
    return nc


def route(x, Wg):
    """Host router identical (up to fp rounding far below the top-2/3
    logit gap) to the reference: top-2 by logit, softmax over the pair."""
    logits = x.astype(np.float32, copy=False) @ Wg.astype(np.float32, copy=False).T
    n = logits.shape[0]
    rows = np.arange(n)
    i1 = np.argmax(logits, axis=1)
    v1 = logits[rows, i1]
    masked = logits.copy()
    masked[rows, i1] = -np.inf
    i2 = np.argmax(masked, axis=1)
    v2 = masked[rows, i2]
    d = np.exp((v2 - v1).astype(np.float64))
    wt1 = (1.0 / (1.0 + d)).astype(np.float32)
    wt2 = (d / (1.0 + d)).astype(np.float32)
    return i1, i2, wt1, wt2


def kernel(x, Wg, W1, b1, W2, b2):
    from concourse.bass_utils import run_bass_kernel_spmd

    x = np.ascontiguousarray(np.asarray(x, dtype=np.float32))
    Wg = np.asarray(Wg, dtype=np.float32)
    W1 = np.asarray(W1, dtype=np.float32)
    b1 = np.asarray(b1, dtype=np.float32)
    W2 = np.asarray(W2, dtype=np.float32)
    b2 = np.asarray(b2, dtype=np.float32)
    n_tokens = x.shape[0]

    i1, i2, wt1, wt2 = route(x, Wg)

    idxs, wts = [], []
    for e in range(N_EXPERTS):
        sel1 = i1 == e
        sel2 = i2 == e
        idx = np.concatenate([np.nonzero(sel1)[0], np.nonzero(sel2)[0]])
        w = np.concatenate([wt1[sel1], wt2[sel2]])
        idxs.append(idx)
        wts.append(w)

    max_count = max(len(i) for i in idxs)
    cap = max(256, -(-max_count // 128) * 128)

    import ml_dtypes

    act_np = ml_dtypes.bfloat16 if MM_MODE == "bf16" else np.float32
    w_np = ml_dtypes.bfloat16 if MM_MODE in ("bf16", "wbf16") else np.float32

    in_maps = []
    for core in range(N_CORES):
        xt = np.zeros((EPC, D_MODEL, cap), dtype=act_np)
        for s in range(EPC):
            e = core * EPC + s
            xt[s, :, : len(idxs[e])] = x[idxs[e]].T.astype(act_np)
        in_maps.append(
            {
                "xt": xt,
                "w1": np.ascontiguousarray(W1[core * EPC : (core + 1) * EPC]).astype(w_np),
                "b1": np.ascontiguousarray(b1[core * EPC : (core + 1) * EPC]),
                "w2": np.ascontiguousarray(W2[core * EPC : (core + 1) * EPC]).astype(w_np),
                "b2": np.ascontiguousarray(b2[core * EPC : (core + 1) * EPC]),
            }
        )

    nc = build_program(cap)
    res = run_bass_kernel_spmd(nc, in_maps, core_ids=list(range(N_CORES)))

    out = np.zeros((n_tokens, D_MODEL), dtype=np.float32)
    for e in range(N_EXPERTS):
        core, s = e // EPC, e % EPC
        n_e = len(idxs[e])
        if n_e == 0:
            continue
        y = res.results[core]["yt"][s, :, :n_e].T  # [n_e, D]
        out[idxs[e]] += wts[e][:, None] * y
    return out


if __name__ == "__main__":
    rng = np.random.default_rng(0)
    x = rng.standard_normal((N_TOKENS, D_MODEL), dtype=np.float32)
    s_in = 1.0 / np.sqrt(D_MODEL)
    s_hid = 1.0 / np.sqrt(D_HIDDEN)
    Wg = rng.uniform(-s_in, s_in, (N_EXPERTS, D_MODEL)).astype(np.float32)
    W1 = rng.uniform(-s_in, s_in, (N_EXPERTS, D_MODEL, D_HIDDEN)).astype(np.float32)
    b1 = rng.uniform(-s_in, s_in, (N_EXPERTS, D_HIDDEN)).astype(np.float32)
    W2 = rng.uniform(-s_hid, s_hid, (N_EXPERTS, D_HIDDEN, D_MODEL)).astype(np.float32)
    b2 = rng.uniform(-s_hid, s_hid, (N_EXPERTS, D_MODEL)).astype(np.float32)
    t0 = time.time()
    out = kernel(x=x, Wg=Wg, W1=W1, b1=b1, W2=W2, b2=b2)
    print("kernel() wall:", time.time() - t0, "out", out.shape, out.dtype)

